# revision 1
# baseline (speedup 1.0000x reference)
"""Trainium2 Bass kernel for the kinematic bicycle-model rollout.

Strategy
--------
The recurrence
    speed_{t+1} = speed_t + DT*clip(a_t)*MAX_ACC
    yaw_{t+1}   = yaw_t + speed_t * tan(clip(s_t))/WHEELBASE * DT
is affine in the start state with batch-independent coefficients, so the
host precomputes (in float64) the [H] vectors
    c[t]  = sum_{i<t} DT*MAX_ACC*clip(a_i)          (speed_t = s0 + c_t)
    A[t]  = sum_{i<t} k_i,  Bv[t] = sum_{i<t} c_i*k_i,
            k_i = tan(clip(s_i))/WHEELBASE*DT       (yaw_t = yaw0 + s0*A_t + Bv_t)
leaving only x/y on-chip work:
    x_t = x0 + sum_{i<t} (DT*c_i + DT*s0) * cos(yaw_i)   (same for y with sin)

Per core (data-parallel over batch, 1024 rollouts/core), batch lives on the
128 SBUF partitions (8 tiles) and time on the free dim (2048):
  - PE:  u_tmp = s0*A/(2pi) + Bv/(2pi)  (contraction-2 matmul into PSUM)
  - ACT: yaw = 2pi*u_tmp + yaw0; speed = cdt/DT + s0; the magic-round affine
         steps t1 = u + MAGIC, kneg = MAGIC - t1 (per-partition scale/bias
         ops); sin/cos via the Sin LUT (valid only on [-pi,pi]):
         sin = Sin(2pi*w), cos = Sin(pi/2 - 2pi*|w|)
  - DVE: w = u + kneg (the wrapped phase in [-.5,.5]);
         v = (cdt + DT*s0)*cos; x/y prefix sums via tensor_tensor_scan
         (the scan runs ~2.1 cyc/elem and is the throughput-limiting op)
Outputs are written batch-major [1024, 2048] (contiguous DMA rows); the host
transposes to the reference's [H, B] layout after gathering the 8 shards.

Measured on trn2 (8 cores): ~165 us HW exec vs ~93 us DMA roofline for the
33.5 MB/core of traffic; DVE (scans + two-tensor ops, ~131 us) and ACT
(7 LUT/affine ops, ~114 us) are the co-bottlenecks, overlapped with DMA.
Work is issued in half-H (1024-col) segments, which cut pipeline stalls by
~20 us vs full-width tiles; quarter-width segments regress (per-op overhead).
"""

import math
import sys

sys.path.insert(0, "/opt/trn_rl_repo")

import numpy as np

import concourse.bacc as bacc
import concourse.mybir as mybir
import concourse.tile as tile
from concourse.bass_utils import run_bass_kernel_spmd

# Model constants (match the reference nn.Module)
H = 2048
B = 8192
NCORES = 8
BL = B // NCORES          # batch per core
P = 128                   # SBUF partitions
NPT = BL // P             # batch tiles per core
DT = 0.05
WHEELBASE = 2.5
MAX_STEER = 0.5
MAX_ACC = 5000.0 / 1000.0

TWO_PI = 2.0 * math.pi
INV_2PI = 1.0 / TWO_PI
HALF_PI = 0.5 * math.pi
MAGIC = 12582912.0        # 1.5 * 2**23: x + MAGIC - MAGIC == round(x) in f32

F32 = mybir.dt.float32
AFT = mybir.ActivationFunctionType
ALU = mybir.AluOpType

_CACHE = {}


def _build():
    nc = bacc.Bacc("TRN2", target_bir_lowering=False, debug=False)

    ab = nc.declare_dram_parameter("ab", [2, H], F32, isOutput=False)
    cdt = nc.declare_dram_parameter("cdt", [H], F32, isOutput=False)
    s0row = nc.declare_dram_parameter("s0row", [BL], F32, isOutput=False)
    cols = nc.declare_dram_parameter("cols", [BL, 5], F32, isOutput=False)
    ox = nc.declare_dram_parameter("ox", [BL, H], F32, isOutput=True)
    oy = nc.declare_dram_parameter("oy", [BL, H], F32, isOutput=True)
    oyaw = nc.declare_dram_parameter("oyaw", [BL, H], F32, isOutput=True)
    ospeed = nc.declare_dram_parameter("ospeed", [BL, H], F32, isOutput=True)

    with tile.TileContext(nc) as tc:
        with (
            tc.tile_pool(name="const", bufs=1) as constp,
            tc.tile_pool(name="io", bufs=3) as iop,
            tc.tile_pool(name="mid", bufs=3) as midp,
            tc.tile_pool(name="psum", bufs=3, space="PSUM") as psp,
        ):
            cdt_bc = constp.tile([P, H], F32)
            nc.sync.dma_start(out=cdt_bc[:], in_=cdt[None, :].to_broadcast((P, H)))
            rhs_ab = constp.tile([2, H], F32)
            nc.sync.dma_start(out=rhs_ab[:], in_=ab[:])
            lhs2 = constp.tile([2, BL], F32)
            nc.vector.memset(lhs2[:], 1.0)
            nc.sync.dma_start(out=lhs2[0:1, :], in_=s0row[None, :])
            halfpi_col = constp.tile([P, 1], F32)
            nc.vector.memset(halfpi_col[:], HALF_PI)
            magic_col = constp.tile([P, 1], F32)
            nc.vector.memset(magic_col[:], MAGIC)

            NSEG = 2
            HH = H // NSEG  # column-segment granularity for tighter pipelining
            for pt in range(NPT):
                sl = slice(pt * P, (pt + 1) * P)
                colt = iop.tile([P, 5], F32, tag="colt")
                nc.sync.dma_start(out=colt[:], in_=cols[sl, :])
                s0_c = colt[:, 0:1]
                sdt0_c = colt[:, 1:2]
                yaw0_c = colt[:, 2:3]
                x0_c = colt[:, 3:4]
                y0_c = colt[:, 4:5]

                x_sb = iop.tile([P, H], F32, tag="x")
                y_sb = iop.tile([P, H], F32, tag="y")
                yaw_sb = iop.tile([P, H], F32, tag="yaw")
                speed_sb = iop.tile([P, H], F32, tag="speed")
                nc.scalar.activation(out=x_sb[:, 0:1], in_=x0_c, func=AFT.Copy)
                nc.scalar.activation(out=y_sb[:, 0:1], in_=y0_c, func=AFT.Copy)

                for hf in range(NSEG):
                    cs = slice(hf * HH, (hf + 1) * HH)

                    # u_tmp = s0*A/(2pi) + Bv/(2pi), on PE (contraction dim 2)
                    ups = psp.tile([P, HH], F32, tag="ups")
                    for j in range(HH // 512):
                        nc.tensor.matmul(
                            ups[:, j * 512 : (j + 1) * 512],
                            lhs2[:, sl],
                            rhs_ab[:, hf * HH + j * 512 : hf * HH + (j + 1) * 512],
                        )

                    nc.scalar.activation(
                        out=yaw_sb[:, cs], in_=ups[:], func=AFT.Identity,
                        bias=yaw0_c, scale=TWO_PI,
                    )
                    nc.scalar.activation(
                        out=speed_sb[:, cs], in_=cdt_bc[:, cs], func=AFT.Identity,
                        bias=s0_c, scale=1.0 / DT,
                    )

                    # w = u - round(u), u = yaw/(2pi); round via the magic
                    # constant (t1/kneg are per-partition affine ops ->
                    # ScalarE; DVE keeps the two-tensor ops + scans)
                    t1 = midp.tile([P, HH], F32, tag="t1")
                    nc.scalar.activation(
                        out=t1[:], in_=yaw_sb[:, cs], func=AFT.Identity,
                        bias=magic_col[:], scale=INV_2PI,
                    )
                    nc.scalar.activation(
                        out=t1[:], in_=t1[:], func=AFT.Identity,
                        bias=magic_col[:], scale=-1.0,
                    )
                    w = midp.tile([P, HH], F32, tag="w")
                    nc.vector.scalar_tensor_tensor(
                        out=w[:], in0=yaw_sb[:, cs], scalar=INV_2PI, in1=t1[:],
                        op0=ALU.mult, op1=ALU.add,
                    )

                    sin_t = midp.tile([P, HH], F32, tag="sin")
                    nc.scalar.activation(out=sin_t[:], in_=w[:], func=AFT.Sin,
                                         scale=TWO_PI)
                    nc.scalar.activation(out=w[:], in_=w[:], func=AFT.Abs)
                    cos_t = midp.tile([P, HH], F32, tag="cos")
                    nc.scalar.activation(
                        out=cos_t[:], in_=w[:], func=AFT.Sin, scale=-TWO_PI,
                        bias=halfpi_col[:],
                    )

                    # v = (cdt + DT*s0) * cos/sin, then prefix-scan over time
                    vx = midp.tile([P, HH], F32, tag="vx")
                    nc.vector.scalar_tensor_tensor(
                        out=vx[:], in0=cdt_bc[:, cs], scalar=sdt0_c, in1=cos_t[:],
                        op0=ALU.add, op1=ALU.mult,
                    )
                    vy = midp.tile([P, HH], F32, tag="vy")
                    nc.vector.scalar_tensor_tensor(
                        out=vy[:], in0=cdt_bc[:, cs], scalar=sdt0_c, in1=sin_t[:],
                        op0=ALU.add, op1=ALU.mult,
                    )

                    # segment s scans v[s*HH .. ] -> x[s*HH+1 ..],
                    # chained off x[s*HH]; the last segment drops v[H-1]
                    lo = hf * HH
                    nd = HH if hf < NSEG - 1 else HH - 1
                    nc.vector.tensor_tensor_scan(
                        out=x_sb[:, lo + 1 : lo + 1 + nd], data0=vx[:, 0:nd],
                        data1=vx[:, 0:nd],
                        initial=(x0_c if hf == 0 else x_sb[:, lo : lo + 1]),
                        op0=ALU.add, op1=ALU.bypass,
                    )
                    nc.vector.tensor_tensor_scan(
                        out=y_sb[:, lo + 1 : lo + 1 + nd], data0=vy[:, 0:nd],
                        data1=vy[:, 0:nd],
                        initial=(y0_c if hf == 0 else y_sb[:, lo : lo + 1]),
                        op0=ALU.add, op1=ALU.bypass,
                    )

                    nc.sync.dma_start(out=ox[sl, cs], in_=x_sb[:, cs])
                    nc.sync.dma_start(out=oy[sl, cs], in_=y_sb[:, cs])
                    nc.sync.dma_start(out=oyaw[sl, cs], in_=yaw_sb[:, cs])
                    nc.sync.dma_start(out=ospeed[sl, cs], in_=speed_sb[:, cs])

    nc.finalize()
    return nc


def _host_precompute(accel, steering):
    a = np.clip(accel.astype(np.float64), -1.0, 1.0)
    dv = DT * MAX_ACC * a
    c = np.concatenate([[0.0], np.cumsum(dv)[: H - 1]])
    st = np.clip(steering.astype(np.float64), -MAX_STEER, MAX_STEER)
    k = np.tan(st) / WHEELBASE * DT
    A = np.concatenate([[0.0], np.cumsum(k)[: H - 1]])
    Bv = np.concatenate([[0.0], np.cumsum(c * k)[: H - 1]])
    ab = (np.stack([A, Bv]) * INV_2PI).astype(np.float32)
    cdt = (DT * c).astype(np.float32)
    return ab, cdt


def _install_ntff_shim():
    """antenv.axon_hooks is absent in this image; recreate it so
    run_bass_kernel_spmd(trace=True) can reach the axon NTFF profiler."""
    import types

    import antenv

    if hasattr(antenv, "axon_hooks"):
        return
    mod = types.ModuleType("antenv.axon_hooks")
    holder = [None]
    mod.set_axon_ntff_profile_hook = lambda h: holder.__setitem__(0, h)
    mod.get_axon_ntff_profile_hook = lambda: holder[0]
    sys.modules["antenv.axon_hooks"] = mod
    antenv.axon_hooks = mod
    from trn_agent_boot.trn_boot import _ntff_profile_via_ctypes

    mod.set_axon_ntff_profile_hook(
        _ntff_profile_via_ctypes("/opt/axon/libaxon_pjrt.so")
    )


def run(start_x, start_y, start_yaw, start_speed, accel, steering, trace=False,
        tmpdir=None):
    if "nc" not in _CACHE:
        _CACHE["nc"] = _build()
    nc = _CACHE["nc"]
    if trace:
        _install_ntff_shim()

    start_x = np.asarray(start_x, dtype=np.float32)
    start_y = np.asarray(start_y, dtype=np.float32)
    start_yaw = np.asarray(start_yaw, dtype=np.float32)
    start_speed = np.asarray(start_speed, dtype=np.float32)
    ab, cdt = _host_precompute(np.asarray(accel), np.asarray(steering))

    in_maps = []
    for i in range(NCORES):
        sl = slice(i * BL, (i + 1) * BL)
        s0 = start_speed[sl]
        cols = np.stack(
            [s0, (DT * s0.astype(np.float64)).astype(np.float32),
             start_yaw[sl], start_x[sl], start_y[sl]],
            axis=1,
        ).astype(np.float32)
        in_maps.append({
            "ab": ab, "cdt": cdt, "s0row": np.ascontiguousarray(s0),
            "cols": np.ascontiguousarray(cols),
        })

    res = run_bass_kernel_spmd(nc, in_maps, core_ids=list(range(NCORES)), trace=trace,
                               tmpdir=tmpdir)

    outs = []
    for key in ("ox", "oy", "oyaw", "ospeed"):
        full = np.concatenate([res.results[i][key] for i in range(NCORES)], axis=0)
        outs.append(np.ascontiguousarray(full.T))
    return tuple(outs), res


def kernel(start_x, start_y, start_yaw, start_speed, accel, steering):
    outs, _ = run(start_x, start_y, start_yaw, start_speed, accel, steering)
    return outs

